# revision 21
# baseline (speedup 1.0000x reference)
"""Trainium2 Bass kernel: chunked causal linear attention (8-core SPMD).

Math (per batch*head pair, chunk-size invariant — global prefix sums):
  Kcs(n)  = sum_{m<=n} k_m          (and rot)
  S(n)    = sum_{m<=n} k_m v_m^T    (and rot)
  den(n)  = q_n.Kcs(n) + qr_n.Kcsrot(n) + eps
  out_n   = (q_n^T S(n) + qr_n^T Srot(n)) / den(n)
Side outputs: Z = Kcs(N-1), S = S(N-1) (and rot).

Device algorithm (v2): recurrence at 128-row block granularity; IO grouped.
Per 512-row chunk (4 blocks):
  - stacked [q|qr], [k|kr] (d-stack 128) PE-transposed to [dstack, m'],
  - per block t: diagonal P = q.k + qr.kr ([128,128], causal-masked via one
    fused tensor-tensor over the chunk), out[t] = maskedP @ [v|1]
    + qqrT[t].T @ [S|z ; Srot|zrot]-snapshot; state += [k|kr].T @ [v|1].
Loads are SWDGE cast-DMAs (fp32->bf16) batched over 8 chunks; matmul operands
bf16, PSUM accumulation fp32.
"""

import numpy as np
import ml_dtypes

B, H, N, D, E = 4, 8, 8192, 64, 64
N_CORES = 8
PAIRS_PER_CORE = (B * H) // N_CORES  # 4
CHUNK = 512
NBLK = CHUNK // 128  # 4
GRP = 4              # chunks per IO group
GBLK = GRP * NBLK    # blocks per IO group

bf16 = ml_dtypes.bfloat16


def _consts():
    ident = np.eye(128, dtype=bf16)
    tri = np.triu(np.ones((128, 128), dtype=np.float32))
    mask4 = np.tile(tri, (1, NBLK)).astype(bf16)   # [128, 512]
    tok = np.zeros((128, 1), dtype=np.float32)
    return ident, mask4, tok


def build_program(n_pairs=PAIRS_PER_CORE, seq=N, repeats=1):
    import contextlib
    import concourse.bacc as bacc
    import concourse.mybir as mybir
    import concourse.tile as tile

    dt = mybir.dt
    nchunks = seq // CHUNK
    grp = min(GRP, nchunks)
    gblk = grp * NBLK
    assert nchunks % grp == 0
    nc = bacc.Bacc("TRN2", target_bir_lowering=False, debug=False,
                   num_devices=N_CORES)

    qd = nc.dram_tensor("q", [n_pairs, seq, D], dt.float32, kind="ExternalInput").ap()
    kd = nc.dram_tensor("k", [n_pairs, seq, D], dt.float32, kind="ExternalInput").ap()
    qrd = nc.dram_tensor("q_rot", [n_pairs, seq, D], dt.float32, kind="ExternalInput").ap()
    krd = nc.dram_tensor("k_rot", [n_pairs, seq, D], dt.float32, kind="ExternalInput").ap()
    vd = nc.dram_tensor("v", [n_pairs, seq, D], dt.float32, kind="ExternalInput").ap()
    identd = nc.dram_tensor("ident", [128, 128], dt.bfloat16, kind="ExternalInput").ap()
    maskd = nc.dram_tensor("mask", [128, 512], dt.bfloat16, kind="ExternalInput").ap()
    tokd = nc.dram_tensor("tok", [128, 1], dt.float32, kind="ExternalInput").ap()

    outd = nc.dram_tensor("out", [n_pairs, seq, E], dt.float32, kind="ExternalOutput").ap()
    Sd = nc.dram_tensor("S", [n_pairs, D, E], dt.float32, kind="ExternalOutput").ap()
    Zd = nc.dram_tensor("Z", [n_pairs, D], dt.float32, kind="ExternalOutput").ap()
    Srd = nc.dram_tensor("S_rot", [n_pairs, D, E], dt.float32, kind="ExternalOutput").ap()
    Zrd = nc.dram_tensor("Z_rot", [n_pairs, D], dt.float32, kind="ExternalOutput").ap()
    tokod = nc.dram_tensor("tok_out", [128, 1], dt.float32, kind="ExternalOutput").ap()

    # [pair, group, row-in-block(128), block-in-group(32), d] DRAM views
    def view(ap):
        return ap.rearrange("a (g b p) d -> a g p b d", b=gblk, p=128)

    qv, kv, qrv, krv, vv, outv = map(view, (qd, kd, qrd, krd, vd, outd))

    Copy = mybir.ActivationFunctionType.Copy

    with tile.TileContext(nc) as tc:
        with (
            tc.tile_pool(name="const", bufs=1) as constp,
            tc.tile_pool(name="qqr", bufs=8) as qqrp,
            tc.tile_pool(name="kkr", bufs=8) as kkrp,
            tc.tile_pool(name="v1", bufs=8) as v1p,
            tc.tile_pool(name="qqrt", bufs=8) as qqrtp,
            tc.tile_pool(name="kkrt", bufs=8) as kkrtp,
            tc.tile_pool(name="pts", bufs=8) as ptsp,
            tc.tile_pool(name="szsb", bufs=16) as szsbp,
            tc.tile_pool(name="outsb", bufs=8) as outsbp,
            tc.tile_pool(name="rcp", bufs=8) as rcpp,
            tc.tile_pool(name="szf", bufs=2) as szfp,
            tc.tile_pool(name="scr", bufs=4, space="PSUM") as scrp,
            tc.tile_pool(name="szp", bufs=4, space="PSUM") as szp,
        ):
            ident = constp.tile([128, 128], dt.bfloat16)
            nc.sync.dma_start(ident[:], identd[:])
            mask4 = constp.tile([128, 512], dt.bfloat16)
            nc.sync.dma_start(mask4[:], maskd[:])
            tokt = constp.tile([128, 1], dt.float32)
            nc.sync.dma_start(tokt[:], tokd[:])
            nc.sync.dma_start(tokod[:], tokt[:])

            group = list(range(n_pairs))
            szps = {}
            for pair in group:
                szps[pair] = szp.tile([128, 65], dt.float32,
                                      name="szacc", tag="szacc")

            rep = (tc.For_i(0, repeats, 1) if repeats > 1
                   else contextlib.nullcontext())
            with rep:
                szsb_prev = {}
                tiles = {}
                for ci in range(nchunks):
                    gi, cg = divmod(ci, grp)
                    for pair in group:
                        if cg == 0:
                            # ---- grouped loads (SWDGE casts fp32->bf16) ----
                            qqr = qqrp.tile([128, gblk, 128], dt.bfloat16,
                                            name="qqr", tag="qqr")
                            nc.gpsimd.dma_start(qqr[:, :, 0:64], qv[pair, gi])
                            nc.gpsimd.dma_start(qqr[:, :, 64:128], qrv[pair, gi])
                            kkr = kkrp.tile([128, gblk, 128], dt.bfloat16,
                                            name="kkr", tag="kkr")
                            nc.gpsimd.dma_start(kkr[:, :, 0:64], kv[pair, gi])
                            nc.gpsimd.dma_start(kkr[:, :, 64:128], krv[pair, gi])
                            v1 = v1p.tile([128, gblk, 65], dt.bfloat16,
                                          name="v1", tag="v1")
                            nc.gpsimd.dma_start(v1[:, :, 0:64], vv[pair, gi])
                            nc.vector.memset(v1[:, :, 64:65], 1.0)
                            outsb = outsbp.tile([128, gblk, 64], dt.float32,
                                                name="outsb", tag="outsb")
                            tiles[pair] = (qqr, kkr, v1, outsb)
                        qqr, kkr, v1, outsb = tiles[pair]

                        # ---- transposes for this chunk's 4 blocks ----
                        tp = scrp.tile([128, 1024], dt.bfloat16, name="tp", tag="scr")
                        for t in range(NBLK):
                            nc.tensor.transpose(tp[:, t * 128:(t + 1) * 128],
                                                qqr[:, cg * NBLK + t, :], ident[:])
                        for t in range(NBLK):
                            nc.tensor.transpose(tp[:, 512 + t * 128:640 + t * 128],
                                                kkr[:, cg * NBLK + t, :], ident[:])
                        qqrt = qqrtp.tile([128, 512], dt.bfloat16, name="qqrt", tag="qqrt")
                        nc.vector.tensor_copy(qqrt[:], tp[:, 0:512])
                        kkrt = kkrtp.tile([128, 512], dt.bfloat16, name="kkrt", tag="kkrt")
                        nc.scalar.copy(kkrt[:], tp[:, 512:1024])

                        # ---- diagonal P blocks + fused causal mask ----
                        ptpa = scrp.tile([128, NBLK, 128], dt.float32, name="ptpa", tag="scr")
                        for t in range(NBLK):
                            nc.tensor.matmul(ptpa[:, t, :],
                                             lhsT=kkrt[:, t * 128:(t + 1) * 128],
                                             rhs=qqrt[:, t * 128:(t + 1) * 128],
                                             start=(t == 0), stop=(t == NBLK - 1))
                        pts = ptsp.tile([128, NBLK, 128], dt.bfloat16, name="pts", tag="pts")
                        nc.vector.tensor_mul(pts[:], ptpa[:], mask4[:])

                        # ---- per-block: out = maskedP @ [v|1] + q.[S|z], state ----
                        outp = scrp.tile([128, NBLK, 65], dt.float32, name="outp", tag="scr")
                        n_mm = 2 * NBLK - (1 if ci == 0 else 0)
                        mm = 0
                        szt = szps[pair]
                        for t in range(NBLK):
                            gb = cg * NBLK + t
                            first_ever = (ci == 0 and t == 0)
                            if not first_ever:
                                nc.tensor.matmul(outp[:, t, :],
                                                 lhsT=qqrt[:, t * 128:(t + 1) * 128],
                                                 rhs=szsb_prev[pair][:],
                                                 start=(mm == 0), stop=(mm == n_mm - 1))
                                mm += 1
                            nc.tensor.matmul(outp[:, t, :],
                                             lhsT=pts[:, t, :],
                                             rhs=v1[:, gb, :],
                                             start=(mm == 0), stop=(mm == n_mm - 1))
                            mm += 1
                            nc.tensor.matmul(
                                szt[:],
                                lhsT=kkr[:, gb, :], rhs=v1[:, gb, :],
                                start=first_ever,
                                stop=(ci == nchunks - 1 and t == NBLK - 1),
                                skip_group_check=True)
                            if ci == nchunks - 1 and t == NBLK - 1:
                                szf = szfp.tile([128, 65], dt.float32, name="szf", tag="szf")
                                nc.scalar.copy(szf[:], szt[:])
                                nc.sync.dma_start(Sd[pair], szf[0:64, 0:64])
                                nc.sync.dma_start(Zd[pair], szf[0:64, 64:65])
                                nc.sync.dma_start(Srd[pair], szf[64:128, 0:64])
                                nc.sync.dma_start(Zrd[pair], szf[64:128, 64:65])
                            else:
                                szsb = szsbp.tile([128, 65], dt.bfloat16,
                                                  name="szsb", tag="szsb")
                                if pair % 2 == 0:
                                    nc.vector.tensor_copy(szsb[:], szt[:])
                                else:
                                    nc.scalar.copy(szsb[:], szt[:])
                                szsb_prev[pair] = szsb

                        # ---- scale by 1/den into grouped store tile ----
                        rcp = rcpp.tile([128, NBLK], dt.float32, name="rcp", tag="rcp")
                        nc.vector.reciprocal(rcp[:], outp[:, :, 64])
                        for t in range(NBLK):
                            eng = nc.vector if t % 2 == 0 else nc.scalar
                            if t % 2 == 0:
                                nc.vector.tensor_scalar_mul(
                                    outsb[:, cg * NBLK + t, :], outp[:, t, 0:64],
                                    rcp[:, t:t + 1])
                            else:
                                nc.scalar.activation(
                                    outsb[:, cg * NBLK + t, :], outp[:, t, 0:64],
                                    Copy, scale=rcp[:, t:t + 1])
                        if cg == grp - 1:
                            nc.sync.dma_start(outv[pair, gi], outsb[:])

    nc.compile()
    return nc


_CACHED = {}


def _get_program(n_pairs=PAIRS_PER_CORE, seq=N):
    key = (n_pairs, seq)
    if key not in _CACHED:
        _CACHED[key] = build_program(n_pairs, seq)
    return _CACHED[key]


def make_in_maps(q, k, q_rot, k_rot, v):
    """Full [B,H,N,D] fp32 arrays -> list of per-core input dicts."""
    ident, mask, tok = _consts()
    flat = {
        "q": np.ascontiguousarray(q.reshape(B * H, N, D), dtype=np.float32),
        "k": np.ascontiguousarray(k.reshape(B * H, N, D), dtype=np.float32),
        "q_rot": np.ascontiguousarray(q_rot.reshape(B * H, N, D), dtype=np.float32),
        "k_rot": np.ascontiguousarray(k_rot.reshape(B * H, N, D), dtype=np.float32),
        "v": np.ascontiguousarray(v.reshape(B * H, N, D), dtype=np.float32),
    }
    in_maps = []
    for c in range(N_CORES):
        sl = slice(c * PAIRS_PER_CORE, (c + 1) * PAIRS_PER_CORE)
        m = {name: np.ascontiguousarray(a[sl]) for name, a in flat.items()}
        m["ident"] = ident
        m["mask"] = mask
        m["tok"] = tok
        in_maps.append(m)
    return in_maps


def assemble(results):
    """Per-core output dicts -> reference-shaped 5-tuple."""
    def cat(name):
        return np.concatenate([np.asarray(r[name]) for r in results], axis=0)

    out = cat("out").reshape(B, H, N, E)
    Z = cat("Z").reshape(B, H, D)
    S = cat("S").reshape(B, H, D, E)
    Zr = cat("Z_rot").reshape(B, H, D)
    Sr = cat("S_rot").reshape(B, H, D, E)
    return out, Z, S, Zr, Sr


def kernel(q, k, q_rot, k_rot, v, chunk_size=None):
    from concourse.bass_utils import run_bass_kernel_spmd

    q, k, q_rot, k_rot, v = (np.asarray(a, dtype=np.float32)
                             for a in (q, k, q_rot, k_rot, v))
    nc = _get_program()
    in_maps = make_in_maps(q, k, q_rot, k_rot, v)
    res = run_bass_kernel_spmd(nc, in_maps, core_ids=list(range(N_CORES)),
                               trace=False)
    return assemble(res.results)


# revision 26
# speedup vs baseline: 1.1023x; 1.1023x over previous
"""Trainium2 Bass kernel: chunked causal linear attention (8-core SPMD).

Math (per batch*head pair, chunk-size invariant — global prefix sums):
  Kcs(n)  = sum_{m<=n} k_m          (and rot)
  S(n)    = sum_{m<=n} k_m v_m^T    (and rot)
  den(n)  = q_n.Kcs(n) + qr_n.Kcsrot(n) + eps
  out_n   = (q_n^T S(n) + qr_n^T Srot(n)) / den(n)
Side outputs: Z = Kcs(N-1), S = S(N-1) (and rot).

Device algorithm (v2): recurrence at 128-row block granularity; IO grouped.
Per 512-row chunk (4 blocks):
  - stacked [q|qr], [k|kr] (d-stack 128) PE-transposed to [dstack, m'],
  - per block t: diagonal P = q.k + qr.kr ([128,128], causal-masked via one
    fused tensor-tensor over the chunk), out[t] = maskedP @ [v|1]
    + qqrT[t].T @ [S|z ; Srot|zrot]-snapshot; state += [k|kr].T @ [v|1].
Loads are SWDGE cast-DMAs (fp32->bf16) batched over 8 chunks; matmul operands
bf16, PSUM accumulation fp32.
"""

import numpy as np
import ml_dtypes

B, H, N, D, E = 4, 8, 8192, 64, 64
N_CORES = 8
PAIRS_PER_CORE = (B * H) // N_CORES  # 4
CHUNK = 512
NBLK = CHUNK // 128  # 4
GRP = 4              # chunks per IO group
GBLK = GRP * NBLK    # blocks per IO group

bf16 = ml_dtypes.bfloat16


def _consts():
    ident = np.eye(128, dtype=bf16)
    tri = np.triu(np.ones((128, 128), dtype=np.float32))
    mask4 = np.tile(tri, (1, NBLK)).astype(bf16)   # [128, 512]
    tok = np.zeros((128, 1), dtype=np.float32)
    return ident, mask4, tok


def build_program(n_pairs=PAIRS_PER_CORE, seq=N, repeats=1):
    import contextlib
    import concourse.bacc as bacc
    import concourse.mybir as mybir
    import concourse.tile as tile

    dt = mybir.dt
    nchunks = seq // CHUNK
    grp = min(GRP, nchunks)
    gblk = grp * NBLK
    assert nchunks % grp == 0
    nc = bacc.Bacc("TRN2", target_bir_lowering=False, debug=False,
                   num_devices=N_CORES)

    qd = nc.dram_tensor("q", [n_pairs, seq, D], dt.float32, kind="ExternalInput").ap()
    kd = nc.dram_tensor("k", [n_pairs, seq, D], dt.float32, kind="ExternalInput").ap()
    qrd = nc.dram_tensor("q_rot", [n_pairs, seq, D], dt.float32, kind="ExternalInput").ap()
    krd = nc.dram_tensor("k_rot", [n_pairs, seq, D], dt.float32, kind="ExternalInput").ap()
    vd = nc.dram_tensor("v", [n_pairs, seq, D], dt.float32, kind="ExternalInput").ap()
    identd = nc.dram_tensor("ident", [128, 128], dt.bfloat16, kind="ExternalInput").ap()
    maskd = nc.dram_tensor("mask", [128, 512], dt.bfloat16, kind="ExternalInput").ap()
    tokd = nc.dram_tensor("tok", [128, 1], dt.float32, kind="ExternalInput").ap()

    outd = nc.dram_tensor("out", [n_pairs, seq, E], dt.float32, kind="ExternalOutput").ap()
    Sd = nc.dram_tensor("S", [n_pairs, D, E], dt.float32, kind="ExternalOutput").ap()
    Zd = nc.dram_tensor("Z", [n_pairs, D], dt.float32, kind="ExternalOutput").ap()
    Srd = nc.dram_tensor("S_rot", [n_pairs, D, E], dt.float32, kind="ExternalOutput").ap()
    Zrd = nc.dram_tensor("Z_rot", [n_pairs, D], dt.float32, kind="ExternalOutput").ap()
    tokod = nc.dram_tensor("tok_out", [128, 1], dt.float32, kind="ExternalOutput").ap()

    # [pair, row-in-block(128), block(seq/128), d] DRAM views
    def view(ap):
        return ap.rearrange("a (b p) d -> a p b d", p=128)

    # chunk index -> (group start chunk, chunks in group): ramp-up then GRP
    groups_sizes = []
    left = nchunks
    while left > 0:
        gs = min(grp, left)
        groups_sizes.append(gs)
        left -= gs
    chunk2group = {}
    c0 = 0
    for gs in groups_sizes:
        for c in range(c0, c0 + gs):
            chunk2group[c] = (c0, gs)
        c0 += gs

    qv, kv, qrv, krv, vv, outv = map(view, (qd, kd, qrd, krd, vd, outd))

    Copy = mybir.ActivationFunctionType.Copy

    with tile.TileContext(nc) as tc:
        with (
            tc.tile_pool(name="const", bufs=1) as constp,
            tc.tile_pool(name="qqr", bufs=8) as qqrp,
            tc.tile_pool(name="kkr", bufs=8) as kkrp,
            tc.tile_pool(name="v1", bufs=8) as v1p,
            tc.tile_pool(name="qqrt", bufs=8) as qqrtp,
            tc.tile_pool(name="kkrt", bufs=8) as kkrtp,
            tc.tile_pool(name="pts", bufs=8) as ptsp,
            tc.tile_pool(name="szsb", bufs=16) as szsbp,
            tc.tile_pool(name="outsb", bufs=8) as outsbp,
            tc.tile_pool(name="rcp", bufs=8) as rcpp,
            tc.tile_pool(name="szf", bufs=2) as szfp,
            tc.tile_pool(name="scr", bufs=4, space="PSUM") as scrp,
            tc.tile_pool(name="szp", bufs=4, space="PSUM") as szp,
        ):
            ident = constp.tile([128, 128], dt.bfloat16)
            nc.sync.dma_start(ident[:], identd[:])
            mask4 = constp.tile([128, 512], dt.bfloat16)
            nc.sync.dma_start(mask4[:], maskd[:])
            tokt = constp.tile([128, 1], dt.float32)
            nc.sync.dma_start(tokt[:], tokd[:])
            nc.sync.dma_start(tokod[:], tokt[:])

            group = list(range(n_pairs))
            szps = {}
            for pair in group:
                szps[pair] = szp.tile([128, 65], dt.float32,
                                      name="szacc", tag="szacc")

            rep = (tc.For_i(0, repeats, 1) if repeats > 1
                   else contextlib.nullcontext())
            with rep:
                szsb_prev = {}
                tiles = {}
                for ci in range(nchunks):
                    g0, gsz = chunk2group[ci]
                    cg = ci - g0
                    for pair in group:
                        if cg == 0:
                            # ---- grouped loads (SWDGE casts fp32->bf16) ----
                            b0, b1 = g0 * NBLK, (g0 + gsz) * NBLK
                            nb = b1 - b0
                            qqr = qqrp.tile([128, gblk, 128], dt.bfloat16,
                                            name="qqr", tag="qqr")
                            nc.gpsimd.dma_start(qqr[:, 0:nb, 0:64], qv[pair, :, b0:b1])
                            nc.gpsimd.dma_start(qqr[:, 0:nb, 64:128], qrv[pair, :, b0:b1])
                            kkr = kkrp.tile([128, gblk, 128], dt.bfloat16,
                                            name="kkr", tag="kkr")
                            nc.gpsimd.dma_start(kkr[:, 0:nb, 0:64], kv[pair, :, b0:b1])
                            nc.gpsimd.dma_start(kkr[:, 0:nb, 64:128], krv[pair, :, b0:b1])
                            v1 = v1p.tile([128, gblk, 65], dt.bfloat16,
                                          name="v1", tag="v1")
                            nc.gpsimd.dma_start(v1[:, 0:nb, 0:64], vv[pair, :, b0:b1])
                            nc.vector.memset(v1[:, 0:nb, 64:65], 1.0)
                            outsb = outsbp.tile([128, gblk, 64], dt.float32,
                                                name="outsb", tag="outsb")
                            tiles[pair] = (qqr, kkr, v1, outsb)
                        qqr, kkr, v1, outsb = tiles[pair]

                        # ---- transposes for this chunk's 4 blocks ----
                        tp = scrp.tile([128, 1024], dt.bfloat16, name="tp", tag="scr")
                        for t in range(NBLK):
                            nc.tensor.transpose(tp[:, t * 128:(t + 1) * 128],
                                                qqr[:, cg * NBLK + t, :], ident[:])
                        for t in range(NBLK):
                            nc.tensor.transpose(tp[:, 512 + t * 128:640 + t * 128],
                                                kkr[:, cg * NBLK + t, :], ident[:])
                        qqrt = qqrtp.tile([128, 512], dt.bfloat16, name="qqrt", tag="qqrt")
                        nc.vector.tensor_copy(qqrt[:], tp[:, 0:512])
                        kkrt = kkrtp.tile([128, 512], dt.bfloat16, name="kkrt", tag="kkrt")
                        nc.scalar.copy(kkrt[:], tp[:, 512:1024])

                        # ---- diagonal P blocks + fused causal mask ----
                        ptpa = scrp.tile([128, NBLK, 128], dt.float32, name="ptpa", tag="scr")
                        for t in range(NBLK):
                            nc.tensor.matmul(ptpa[:, t, :],
                                             lhsT=kkrt[:, t * 128:(t + 1) * 128],
                                             rhs=qqrt[:, t * 128:(t + 1) * 128],
                                             start=(t == 0), stop=(t == NBLK - 1))
                        pts = ptsp.tile([128, NBLK, 128], dt.bfloat16, name="pts", tag="pts")
                        nc.vector.tensor_mul(pts[:], ptpa[:], mask4[:])

                        # ---- per-block: out = maskedP @ [v|1] + q.[S|z], state ----
                        outp = scrp.tile([128, NBLK, 65], dt.float32, name="outp", tag="scr")
                        n_mm = 2 * NBLK - (1 if ci == 0 else 0)
                        mm = 0
                        szt = szps[pair]
                        for t in range(NBLK):
                            gb = cg * NBLK + t
                            first_ever = (ci == 0 and t == 0)
                            if not first_ever:
                                nc.tensor.matmul(outp[:, t, :],
                                                 lhsT=qqrt[:, t * 128:(t + 1) * 128],
                                                 rhs=szsb_prev[pair][:],
                                                 start=(mm == 0), stop=(mm == n_mm - 1))
                                mm += 1
                            nc.tensor.matmul(outp[:, t, :],
                                             lhsT=pts[:, t, :],
                                             rhs=v1[:, gb, :],
                                             start=(mm == 0), stop=(mm == n_mm - 1))
                            mm += 1
                            nc.tensor.matmul(
                                szt[:],
                                lhsT=kkr[:, gb, :], rhs=v1[:, gb, :],
                                start=first_ever,
                                stop=(ci == nchunks - 1 and t == NBLK - 1),
                                skip_group_check=True)
                            if ci == nchunks - 1 and t == NBLK - 1:
                                szf = szfp.tile([128, 65], dt.float32, name="szf", tag="szf")
                                nc.scalar.copy(szf[:], szt[:])
                                nc.sync.dma_start(Sd[pair], szf[0:64, 0:64])
                                nc.sync.dma_start(Zd[pair], szf[0:64, 64:65])
                                nc.sync.dma_start(Srd[pair], szf[64:128, 0:64])
                                nc.sync.dma_start(Zrd[pair], szf[64:128, 64:65])
                            else:
                                szsb = szsbp.tile([128, 65], dt.bfloat16,
                                                  name="szsb", tag="szsb")
                                if pair % 2 == 0:
                                    nc.vector.tensor_copy(szsb[:], szt[:])
                                else:
                                    nc.scalar.copy(szsb[:], szt[:])
                                szsb_prev[pair] = szsb

                        # ---- scale by 1/den into grouped store tile ----
                        rcp = rcpp.tile([128, NBLK], dt.float32, name="rcp", tag="rcp")
                        nc.vector.reciprocal(rcp[:], outp[:, :, 64])
                        for t in range(NBLK):
                            eng = nc.vector if t % 2 == 0 else nc.scalar
                            if t % 2 == 0:
                                nc.vector.tensor_scalar_mul(
                                    outsb[:, cg * NBLK + t, :], outp[:, t, 0:64],
                                    rcp[:, t:t + 1])
                            else:
                                nc.scalar.activation(
                                    outsb[:, cg * NBLK + t, :], outp[:, t, 0:64],
                                    Copy, scale=rcp[:, t:t + 1])
                        if cg == gsz - 1:
                            nc.sync.dma_start(
                                outv[pair, :, g0 * NBLK:(g0 + gsz) * NBLK],
                                outsb[:, 0:(gsz * NBLK), :])

    nc.compile()
    return nc


_CACHED = {}


def _get_program(n_pairs=PAIRS_PER_CORE, seq=N):
    key = (n_pairs, seq)
    if key not in _CACHED:
        _CACHED[key] = build_program(n_pairs, seq)
    return _CACHED[key]


def make_in_maps(q, k, q_rot, k_rot, v):
    """Full [B,H,N,D] fp32 arrays -> list of per-core input dicts."""
    ident, mask, tok = _consts()
    flat = {
        "q": np.ascontiguousarray(q.reshape(B * H, N, D), dtype=np.float32),
        "k": np.ascontiguousarray(k.reshape(B * H, N, D), dtype=np.float32),
        "q_rot": np.ascontiguousarray(q_rot.reshape(B * H, N, D), dtype=np.float32),
        "k_rot": np.ascontiguousarray(k_rot.reshape(B * H, N, D), dtype=np.float32),
        "v": np.ascontiguousarray(v.reshape(B * H, N, D), dtype=np.float32),
    }
    in_maps = []
    for c in range(N_CORES):
        sl = slice(c * PAIRS_PER_CORE, (c + 1) * PAIRS_PER_CORE)
        m = {name: np.ascontiguousarray(a[sl]) for name, a in flat.items()}
        m["ident"] = ident
        m["mask"] = mask
        m["tok"] = tok
        in_maps.append(m)
    return in_maps


def assemble(results):
    """Per-core output dicts -> reference-shaped 5-tuple."""
    def cat(name):
        return np.concatenate([np.asarray(r[name]) for r in results], axis=0)

    out = cat("out").reshape(B, H, N, E)
    Z = cat("Z").reshape(B, H, D)
    S = cat("S").reshape(B, H, D, E)
    Zr = cat("Z_rot").reshape(B, H, D)
    Sr = cat("S_rot").reshape(B, H, D, E)
    return out, Z, S, Zr, Sr


def kernel(q, k, q_rot, k_rot, v, chunk_size=None):
    from concourse.bass_utils import run_bass_kernel_spmd

    q, k, q_rot, k_rot, v = (np.asarray(a, dtype=np.float32)
                             for a in (q, k, q_rot, k_rot, v))
    nc = _get_program()
    in_maps = make_in_maps(q, k, q_rot, k_rot, v)
    res = run_bass_kernel_spmd(nc, in_maps, core_ids=list(range(N_CORES)),
                               trace=False)
    return assemble(res.results)


# revision 30
# speedup vs baseline: 1.3388x; 1.2145x over previous
"""Trainium2 Bass kernel: chunked causal linear attention (8-core SPMD).

Math (per batch*head pair, chunk-size invariant — global prefix sums):
  Kcs(n)  = sum_{m<=n} k_m          (and rot)
  S(n)    = sum_{m<=n} k_m v_m^T    (and rot)
  den(n)  = q_n.Kcs(n) + qr_n.Kcsrot(n) + eps
  out_n   = (q_n^T S(n) + qr_n^T Srot(n)) / den(n)
Side outputs: Z = Kcs(N-1), S = S(N-1) (and rot).

Device algorithm: recurrence at 128-row block granularity, all 4 pairs of a
core interleaved (4 independent dependency chains hide the state->snapshot->
inter latency). Per 512-row chunk (4 blocks):
  - stacked [q|qr], [k|kr] (d-stack of 128) PE-transposed to [dstack, m'],
  - per block t: diagonal P = q.k + qr.kr ([128,128]; causal mask applied by
    one fused tensor-tensor mul over the whole chunk), then
    out[t] = maskedP @ [v|1] + qqrT[t].T @ [S|z ; Srot|zrot]-snapshot,
    state(psum) += [k|kr].T @ [v|1], snapshot = copy(state) -> SBUF bf16.
  - denominators ride along as column 64 ([v|1] / [S|z]); one reciprocal +
    per-block scale produce the output tile.
IO: q/q_rot and k/k_rot are host-concatenated per 128-row block so each is a
single SWDGE cast-DMA (fp32->bf16) batched over 4 chunks; matmul operands are
bf16, all PSUM accumulation fp32. PSUM: 4 state banks + 4 shared scratch
banks (transpose staging / P / out rotate through one pool).
"""

import numpy as np
import ml_dtypes

B, H, N, D, E = 4, 8, 8192, 64, 64
N_CORES = 8
PAIRS_PER_CORE = (B * H) // N_CORES  # 4
CHUNK = 512
NBLK = CHUNK // 128  # 4
GRP = 4              # chunks per IO group
GBLK = GRP * NBLK    # blocks per IO group

bf16 = ml_dtypes.bfloat16


def _consts():
    ident = np.eye(128, dtype=bf16)
    tri = np.triu(np.ones((128, 128), dtype=np.float32))
    mask4 = np.tile(tri, (1, NBLK)).astype(bf16)   # [128, 512]
    tok = np.zeros((128, 1), dtype=np.float32)
    return ident, mask4, tok


def build_program(n_pairs=PAIRS_PER_CORE, seq=N, repeats=1):
    import contextlib
    import concourse.bacc as bacc
    import concourse.mybir as mybir
    import concourse.tile as tile

    dt = mybir.dt
    nchunks = seq // CHUNK
    grp = min(GRP, nchunks)
    gblk = grp * NBLK
    assert nchunks % grp == 0
    nc = bacc.Bacc("TRN2", target_bir_lowering=False, debug=False,
                   num_devices=N_CORES)

    nblocks = seq // 128
    qqrd = nc.dram_tensor("qqr_cat", [n_pairs, nblocks, 2, 128, D], dt.float32, kind="ExternalInput").ap()
    kkrd = nc.dram_tensor("kkr_cat", [n_pairs, nblocks, 2, 128, D], dt.float32, kind="ExternalInput").ap()
    vd = nc.dram_tensor("v", [n_pairs, seq, D], dt.float32, kind="ExternalInput").ap()
    identd = nc.dram_tensor("ident", [128, 128], dt.bfloat16, kind="ExternalInput").ap()
    maskd = nc.dram_tensor("mask", [128, 512], dt.bfloat16, kind="ExternalInput").ap()
    tokd = nc.dram_tensor("tok", [128, 1], dt.float32, kind="ExternalInput").ap()

    outd = nc.dram_tensor("out", [n_pairs, seq, E], dt.float32, kind="ExternalOutput").ap()
    Sd = nc.dram_tensor("S", [n_pairs, D, E], dt.float32, kind="ExternalOutput").ap()
    Zd = nc.dram_tensor("Z", [n_pairs, D], dt.float32, kind="ExternalOutput").ap()
    Srd = nc.dram_tensor("S_rot", [n_pairs, D, E], dt.float32, kind="ExternalOutput").ap()
    Zrd = nc.dram_tensor("Z_rot", [n_pairs, D], dt.float32, kind="ExternalOutput").ap()
    tokod = nc.dram_tensor("tok_out", [128, 1], dt.float32, kind="ExternalOutput").ap()

    # [pair, row-in-block(128), block(seq/128), d] DRAM views
    def view(ap):
        return ap.rearrange("a (b p) d -> a p b d", p=128)

    def viewcat(ap):
        # [pair, block, which(2), p, d] -> [pair, p, block, which, d]
        return ap.rearrange("a b w p d -> a p b w d")

    # chunk index -> (group start chunk, chunks in group): ramp-up then GRP
    groups_sizes = []
    left = nchunks
    while left > 0:
        gs = min(grp, left)
        groups_sizes.append(gs)
        left -= gs
    chunk2group = {}
    c0 = 0
    for gs in groups_sizes:
        for c in range(c0, c0 + gs):
            chunk2group[c] = (c0, gs)
        c0 += gs

    qqrv, kkrv = map(viewcat, (qqrd, kkrd))
    vv, outv = map(view, (vd, outd))

    Copy = mybir.ActivationFunctionType.Copy

    with tile.TileContext(nc) as tc:
        with (
            tc.tile_pool(name="const", bufs=1) as constp,
            tc.tile_pool(name="qqr", bufs=8) as qqrp,
            tc.tile_pool(name="kkr", bufs=8) as kkrp,
            tc.tile_pool(name="v1", bufs=8) as v1p,
            tc.tile_pool(name="qqrt", bufs=8) as qqrtp,
            tc.tile_pool(name="kkrt", bufs=8) as kkrtp,
            tc.tile_pool(name="pts", bufs=8) as ptsp,
            tc.tile_pool(name="szsb", bufs=16) as szsbp,
            tc.tile_pool(name="outsb", bufs=8) as outsbp,
            tc.tile_pool(name="rcp", bufs=8) as rcpp,
            tc.tile_pool(name="szf", bufs=2) as szfp,
            tc.tile_pool(name="scr", bufs=4, space="PSUM") as scrp,
            tc.tile_pool(name="szp", bufs=4, space="PSUM") as szp,
        ):
            ident = constp.tile([128, 128], dt.bfloat16)
            nc.sync.dma_start(ident[:], identd[:])
            mask4 = constp.tile([128, 512], dt.bfloat16)
            nc.sync.dma_start(mask4[:], maskd[:])
            tokt = constp.tile([128, 1], dt.float32)
            nc.sync.dma_start(tokt[:], tokd[:])
            nc.sync.dma_start(tokod[:], tokt[:])

            group = list(range(n_pairs))
            szps = {}
            for pair in group:
                szps[pair] = szp.tile([128, 65], dt.float32,
                                      name="szacc", tag="szacc")

            rep = (tc.For_i(0, repeats, 1) if repeats > 1
                   else contextlib.nullcontext())
            with rep:
                szsb_prev = {}
                tiles = {}
                for ci in range(nchunks):
                    g0, gsz = chunk2group[ci]
                    cg = ci - g0
                    for pair in group:
                        if cg == 0:
                            # ---- grouped loads (SWDGE casts fp32->bf16) ----
                            b0, b1 = g0 * NBLK, (g0 + gsz) * NBLK
                            nb = b1 - b0
                            qqr = qqrp.tile([128, gblk, 2, 64], dt.bfloat16,
                                            name="qqr", tag="qqr")
                            nc.gpsimd.dma_start(qqr[:, 0:nb], qqrv[pair, :, b0:b1])
                            kkr = kkrp.tile([128, gblk, 2, 64], dt.bfloat16,
                                            name="kkr", tag="kkr")
                            nc.gpsimd.dma_start(kkr[:, 0:nb], kkrv[pair, :, b0:b1])
                            v1 = v1p.tile([128, gblk, 65], dt.bfloat16,
                                          name="v1", tag="v1")
                            nc.gpsimd.dma_start(v1[:, 0:nb, 0:64], vv[pair, :, b0:b1])
                            nc.vector.memset(v1[:, 0:nb, 64:65], 1.0)
                            outsb = outsbp.tile([128, gblk, 64], dt.float32,
                                                name="outsb", tag="outsb")
                            tiles[pair] = (qqr, kkr, v1, outsb)
                        qqr, kkr, v1, outsb = tiles[pair]

                        # ---- transposes for this chunk's 4 blocks ----
                        tp = scrp.tile([128, 1024], dt.bfloat16, name="tp", tag="scr")
                        for t in range(NBLK):
                            nc.tensor.transpose(tp[:, t * 128:(t + 1) * 128],
                                                qqr[:, cg * NBLK + t], ident[:])
                        for t in range(NBLK):
                            nc.tensor.transpose(tp[:, 512 + t * 128:640 + t * 128],
                                                kkr[:, cg * NBLK + t], ident[:])
                        qqrt = qqrtp.tile([128, 512], dt.bfloat16, name="qqrt", tag="qqrt")
                        nc.vector.tensor_copy(qqrt[:], tp[:, 0:512])
                        kkrt = kkrtp.tile([128, 512], dt.bfloat16, name="kkrt", tag="kkrt")
                        nc.scalar.copy(kkrt[:], tp[:, 512:1024])

                        # ---- diagonal P blocks + fused causal mask ----
                        ptpa = scrp.tile([128, NBLK, 128], dt.float32, name="ptpa", tag="scr")
                        for t in range(NBLK):
                            nc.tensor.matmul(ptpa[:, t, :],
                                             lhsT=kkrt[:, t * 128:(t + 1) * 128],
                                             rhs=qqrt[:, t * 128:(t + 1) * 128],
                                             start=(t == 0), stop=(t == NBLK - 1))
                        pts = ptsp.tile([128, NBLK, 128], dt.bfloat16, name="pts", tag="pts")
                        nc.vector.tensor_mul(pts[:], ptpa[:], mask4[:])

                        # ---- per-block: out = maskedP @ [v|1] + q.[S|z], state ----
                        outp = scrp.tile([128, NBLK, 65], dt.float32, name="outp", tag="scr")
                        n_mm = 2 * NBLK - (1 if ci == 0 else 0)
                        mm = 0
                        szt = szps[pair]
                        for t in range(NBLK):
                            gb = cg * NBLK + t
                            first_ever = (ci == 0 and t == 0)
                            if not first_ever:
                                nc.tensor.matmul(outp[:, t, :],
                                                 lhsT=qqrt[:, t * 128:(t + 1) * 128],
                                                 rhs=szsb_prev[pair][:],
                                                 start=(mm == 0), stop=(mm == n_mm - 1))
                                mm += 1
                            nc.tensor.matmul(outp[:, t, :],
                                             lhsT=pts[:, t, :],
                                             rhs=v1[:, gb, :],
                                             start=(mm == 0), stop=(mm == n_mm - 1))
                            mm += 1
                            nc.tensor.matmul(
                                szt[:],
                                lhsT=kkr[:, gb], rhs=v1[:, gb, :],
                                start=first_ever,
                                stop=(ci == nchunks - 1 and t == NBLK - 1),
                                skip_group_check=True)
                            if ci == nchunks - 1 and t == NBLK - 1:
                                szf = szfp.tile([128, 65], dt.float32, name="szf", tag="szf")
                                nc.scalar.copy(szf[:], szt[:])
                                nc.sync.dma_start(Sd[pair], szf[0:64, 0:64])
                                nc.sync.dma_start(Zd[pair], szf[0:64, 64:65])
                                nc.sync.dma_start(Srd[pair], szf[64:128, 0:64])
                                nc.sync.dma_start(Zrd[pair], szf[64:128, 64:65])
                            else:
                                szsb = szsbp.tile([128, 65], dt.bfloat16,
                                                  name="szsb", tag="szsb")
                                if pair % 2 == 0:
                                    nc.vector.tensor_copy(szsb[:], szt[:])
                                else:
                                    nc.scalar.copy(szsb[:], szt[:])
                                szsb_prev[pair] = szsb

                        # ---- scale by 1/den into grouped store tile ----
                        rcp = rcpp.tile([128, NBLK], dt.float32, name="rcp", tag="rcp")
                        nc.vector.reciprocal(rcp[:], outp[:, :, 64])
                        for t in range(NBLK):
                            eng = nc.vector if t % 2 == 0 else nc.scalar
                            if t % 2 == 0:
                                nc.vector.tensor_scalar_mul(
                                    outsb[:, cg * NBLK + t, :], outp[:, t, 0:64],
                                    rcp[:, t:t + 1])
                            else:
                                nc.scalar.activation(
                                    outsb[:, cg * NBLK + t, :], outp[:, t, 0:64],
                                    Copy, scale=rcp[:, t:t + 1])
                        if cg == gsz - 1:
                            nc.sync.dma_start(
                                outv[pair, :, g0 * NBLK:(g0 + gsz) * NBLK],
                                outsb[:, 0:(gsz * NBLK), :])

    nc.compile()
    return nc


_CACHED = {}


def _get_program(n_pairs=PAIRS_PER_CORE, seq=N):
    key = (n_pairs, seq)
    if key not in _CACHED:
        _CACHED[key] = build_program(n_pairs, seq)
    return _CACHED[key]


def make_in_maps(q, k, q_rot, k_rot, v):
    """Full [B,H,N,D] fp32 arrays -> list of per-core input dicts."""
    ident, mask, tok = _consts()
    nb = N // 128
    qqr_cat = np.stack([q.reshape(B * H, nb, 128, D),
                        q_rot.reshape(B * H, nb, 128, D)], axis=2).astype(np.float32)
    kkr_cat = np.stack([k.reshape(B * H, nb, 128, D),
                        k_rot.reshape(B * H, nb, 128, D)], axis=2).astype(np.float32)
    flat = {
        "qqr_cat": qqr_cat,
        "kkr_cat": kkr_cat,
        "v": np.ascontiguousarray(v.reshape(B * H, N, D), dtype=np.float32),
    }
    in_maps = []
    for c in range(N_CORES):
        sl = slice(c * PAIRS_PER_CORE, (c + 1) * PAIRS_PER_CORE)
        m = {name: np.ascontiguousarray(a[sl]) for name, a in flat.items()}
        m["ident"] = ident
        m["mask"] = mask
        m["tok"] = tok
        in_maps.append(m)
    return in_maps


def assemble(results):
    """Per-core output dicts -> reference-shaped 5-tuple."""
    def cat(name):
        return np.concatenate([np.asarray(r[name]) for r in results], axis=0)

    out = cat("out").reshape(B, H, N, E)
    Z = cat("Z").reshape(B, H, D)
    S = cat("S").reshape(B, H, D, E)
    Zr = cat("Z_rot").reshape(B, H, D)
    Sr = cat("S_rot").reshape(B, H, D, E)
    return out, Z, S, Zr, Sr


def kernel(q, k, q_rot, k_rot, v, chunk_size=None):
    from concourse.bass_utils import run_bass_kernel_spmd

    q, k, q_rot, k_rot, v = (np.asarray(a, dtype=np.float32)
                             for a in (q, k, q_rot, k_rot, v))
    nc = _get_program()
    in_maps = make_in_maps(q, k, q_rot, k_rot, v)
    res = run_bass_kernel_spmd(nc, in_maps, core_ids=list(range(N_CORES)),
                               trace=False)
    return assemble(res.results)
